# revision 7
# baseline (speedup 1.0000x reference)
"""Trainium2 Bass kernel for space-to-depth (pixel-unshuffle, factor 2).

Input  x:   (8, 32, 512, 512) f32
Output out: (8, 128, 256, 256) f32 with out[b, 4i+2dh+dw, h, w] = x[b, i, 2h+dh, 2w+dw]

Sharding: data-parallel over batch -- core b processes sample b (no comms).

Per-core dataflow (v2): 8 groups of 4 input channels (4MB tiles).
  - one 4MB load per group (32KB contiguous per partition; SP HWDGE ring)
  - 4 strided DVE copies (one per (dh,dw) phase) deinterleave in SBUF;
    fully hidden behind DMA
  - 4 stores of 1MB per group (8KB contiguous runs; ACT HWDGE ring;
    single_packet=True bundles each engine's descriptors into one packet,
    coarsening R/W interleave -- won 3/4 paired A/Bs, mean +4.6us/op)
Measured on TRN2 (8 cores): ~203-210us exec; steady-state marginal rate
~190us/op = 354 GB/s combined R+W per core (pure-read 399, pure-write 374,
interleaved-mix 313 GB/s measured), ~8us framework preamble + ~3us tail.
"""

import numpy as np

from concourse import bacc, mybir, tile
from concourse.bass_utils import run_bass_kernel_spmd

B, C, H, W = 8, 32, 512, 512
N_CORES = 8

_cache = {}


def _build_nc(finalize=True, reps=1, variant="v2", bufs=(3, 2), store_engine="scalar", store_sp=True, load_sp=False, load_split=1):
    nc = bacc.Bacc(
        "TRN2", target_bir_lowering=False, debug=False, num_devices=N_CORES
    )
    x = nc.dram_tensor("x", [C, H, W], mybir.dt.float32, kind="ExternalInput")
    out = nc.dram_tensor(
        "out", [4 * C, H // 2, W // 2], mybir.dt.float32, kind="ExternalOutput"
    )
    xa, oa = x.ap(), out.ap()

    if variant == "raw":
        _emit_raw(nc, xa, oa, reps)
    else:
        with tile.TileContext(nc) as tc:
            if variant == "v1":
                _emit_v1(nc, tc, xa, oa, reps)
            elif variant == "v3":
                _emit_v3(nc, tc, xa, oa, reps, bufs, store_engine, store_sp, load_sp, load_split)
            else:
                _emit_v2(nc, tc, xa, oa, reps, bufs, store_engine, store_sp, load_sp)
    if finalize:
        nc.finalize()
    return nc


def _emit_v3(nc, tc, xa, oa, reps, bufs=(2, 4), store_engine="scalar", store_sp=True, load_sp=False, load_split=1):
    """8 channels per tile (8MB); per-channel DMAs with DRAM-side outer dim
    16 so every dma_start fans across all 16 SDMA engines.

    The engine fan of a dma_start follows the DRAM-side outer AP dim
    (min(outer, 16)): loads are one dma per (g, ci) with DRAM AP
    (16, 16384/load_split), stores one dma per (g, ci, co) writing one FULL
    output channel (256KB contiguous) with DRAM AP (16, 4096) -- 16 engines
    x one 16KB descriptor each.

    In-tile partition p=(ci=p>>4, pp=p&15) holds x[8g+ci, 32pp:32pp+32, :]
    (64KB contiguous).  Staging tile s_co (co=2dh+dw) partition p holds
    out[32g+4ci+co, 16pp:16pp+16, :] as one 16KB contiguous run.
    load_split subdivides each 1MB channel load along the free dim to
    shrink load packets (64KB/engine /split) for R/W service balance.
    """
    G = C // 8  # 4 groups
    if isinstance(bufs, int):
        bufs = (bufs, bufs)
    store_eng = getattr(nc, store_engine)
    with (
        tc.tile_pool(name="inp", bufs=bufs[0]) as ip,
        tc.tile_pool(name="stg", bufs=bufs[1]) as sp,
    ):
        for _ in range(reps):
            for g in range(G):
                t = ip.tile([128, 16384], mybir.dt.float32)
                for ci in range(8):
                    src = xa[8 * g + ci].rearrange(
                        "(pp r) w -> pp (r w)", pp=16
                    )
                    dst = t[16 * ci : 16 * ci + 16]
                    if load_split == 1:
                        nc.sync.dma_start(dst, src, single_packet=load_sp)
                    else:
                        sv = src.rearrange("p (ls q) -> ls p q", ls=load_split)
                        dv = dst.rearrange("p (ls q) -> ls p q", ls=load_split)
                        for j in range(load_split):
                            nc.sync.dma_start(dv[j], sv[j], single_packet=load_sp)
                t3 = t[:].rearrange("p (r w) -> p r w", r=32)
                for co in range(4):
                    dh, dw = co >> 1, co & 1
                    s = sp.tile([128, 4096], mybir.dt.float32)
                    nc.vector.tensor_copy(
                        s[:].rearrange("p (hh w) -> p hh w", hh=16),
                        t3[:, dh::2, dw::2],
                    )
                    for ci in range(8):
                        store_eng.dma_start(
                            oa[32 * g + 4 * ci + co].rearrange(
                                "(pp hh) w -> pp (hh w)", pp=16
                            ),
                            s[16 * ci : 16 * ci + 16],
                            single_packet=store_sp,
                        )


def _emit_raw(nc, xa, oa, reps):
    """Same dataflow as v2 but raw bacc: hand-rolled semaphore pipeline,
    no TileContext, so the first load issues immediately instead of after
    the ~8us Tile preamble.  3 in-tile buffers, 2 staging buffers.
    """
    G = C // 4
    NB_IN, NB_ST = 3, 2
    tin = [
        nc.alloc_sbuf_tensor(f"tin{j}", [128, 8192], mybir.dt.float32)
        for j in range(NB_IN)
    ]
    tst = [
        nc.alloc_sbuf_tensor(f"tst{j}", [128, 8192], mybir.dt.float32)
        for j in range(NB_ST)
    ]
    n = G * reps
    from contextlib import ExitStack

    with ExitStack() as ctx:
        block = ctx.enter_context(nc.Block())
        # per-buffer sem rotation so concurrent DMAs never share a semaphore
        ld_sems = [
            ctx.enter_context(nc.semaphore(f"ld_sem{j}")) for j in range(NB_IN)
        ]
        st_sems = [
            ctx.enter_context(nc.semaphore(f"st_sem{j}")) for j in range(NB_ST)
        ]
        cp_sem = ctx.enter_context(nc.semaphore("cp_sem"))

        @block.sync
        def _(sync):
            for k in range(n):
                g = k % G
                if k >= NB_IN:
                    # in-buffer reuse: copies of group k-NB_IN must be done
                    sync.wait_ge(cp_sem, k - NB_IN + 1)
                sync.dma_start(
                    tin[k % NB_IN].ap(),
                    xa[4 * g : 4 * g + 4].rearrange(
                        "ci (pp r) w -> (ci pp) (r w)", pp=32
                    ),
                ).then_inc(ld_sems[k % NB_IN], 16)

        @block.vector
        def _(vec):
            for k in range(n):
                vec.wait_ge(ld_sems[k % NB_IN], 16 * (k // NB_IN + 1))
                if k >= NB_ST:
                    # staging reuse: stores of group k-NB_ST must be done
                    vec.wait_ge(st_sems[k % NB_ST], 64 * (k // NB_ST))
                t3 = tin[k % NB_IN].ap().rearrange("p (j w) -> p j w", j=16)
                s4 = (
                    tst[k % NB_ST]
                    .ap()
                    .rearrange("p (co hh w) -> p co hh w", co=4, hh=8)
                )
                last = None
                for dh in range(2):
                    for dw in range(2):
                        last = vec.tensor_copy(
                            s4[:, 2 * dh + dw], t3[:, dh::2, dw::2]
                        )
                last.then_inc(cp_sem, 1)

        @block.scalar
        def _(scalar):
            for k in range(n):
                g = k % G
                scalar.wait_ge(cp_sem, k + 1)
                s = tst[k % NB_ST].ap()
                for ci in range(4):
                    c0 = 16 * g + 4 * ci
                    scalar.dma_start(
                        oa[c0 : c0 + 4].rearrange(
                            "co (pp hh) w -> pp co (hh w)", hh=8
                        ),
                        s[32 * ci : 32 * ci + 32].rearrange(
                            "p (co q) -> p co q", co=4
                        ),
                    ).then_inc(st_sems[k % NB_ST], 16)


def _emit_v1(nc, tc, xa, oa, reps):
    """1 channel per tile: 1MB loads (8KB descs), 1MB stores (2KB descs)."""
    with (
        tc.tile_pool(name="inp", bufs=3) as ip,
        tc.tile_pool(name="stg", bufs=3) as sp,
    ):
        for _ in range(reps):
            for i in range(C):
                t = ip.tile([128, 2048], mybir.dt.float32)
                # partition p <- x[i, 4p:4p+4, :] (8KB contiguous per partition)
                nc.sync.dma_start(
                    t[:], xa[i].rearrange("(p r) w -> p (r w)", p=128)
                )
                s = sp.tile([128, 2048], mybir.dt.float32)
                t3 = t[:].rearrange("p (j w) -> p j w", j=4)
                s4 = s[:].rearrange("p (c hh w) -> p c hh w", c=4, hh=2)
                for dh in range(2):
                    for dw in range(2):
                        nc.vector.tensor_copy(
                            s4[:, 2 * dh + dw], t3[:, dh::2, dw::2]
                        )
                # staging partition p rows (2p, 2p+1) -> 2KB contiguous runs
                nc.sync.dma_start(
                    oa[4 * i : 4 * i + 4].rearrange(
                        "c (p hh) w -> p c (hh w)", p=128, hh=2
                    ),
                    s[:].rearrange("p (c q) -> p c q", c=4),
                )


def _emit_v2(nc, tc, xa, oa, reps, bufs, store_engine="scalar", store_sp=False, load_sp=False):
    """4 channels per tile (4MB): 8KB descriptors on BOTH load and store;
    loads on the SP HWDGE ring, stores on the ACT ring.

    Tile partition p = (ci=p>>5, pp=p&31) holds x[4g+ci, 16pp:16pp+16, :]
    (32KB contiguous).  Staging partition p holds, for each co in 0..3,
    out[4*(4g+ci)+co, 8pp:8pp+8, :] as one 8KB contiguous run.
    """
    G = C // 4  # 8 groups
    if isinstance(bufs, int):
        bufs = (bufs, bufs)
    store_eng = getattr(nc, store_engine)
    with (
        tc.tile_pool(name="inp", bufs=bufs[0]) as ip,
        tc.tile_pool(name="stg", bufs=bufs[1]) as sp,
    ):
        for _ in range(reps):
            for g in range(G):
                t = ip.tile([128, 8192], mybir.dt.float32)
                nc.sync.dma_start(
                    t[:],
                    xa[4 * g : 4 * g + 4].rearrange(
                        "ci (pp r) w -> (ci pp) (r w)", pp=32
                    ),
                    single_packet=load_sp,
                )
                s = sp.tile([128, 8192], mybir.dt.float32)
                t3 = t[:].rearrange("p (j w) -> p j w", j=16)
                s4 = s[:].rearrange("p (co hh w) -> p co hh w", co=4, hh=8)
                for dh in range(2):
                    for dw in range(2):
                        nc.vector.tensor_copy(
                            s4[:, 2 * dh + dw], t3[:, dh::2, dw::2]
                        )
                for ci in range(4):
                    c0 = 16 * g + 4 * ci
                    store_eng.dma_start(
                        oa[c0 : c0 + 4].rearrange(
                            "co (pp hh) w -> pp co (hh w)", hh=8
                        ),
                        s[32 * ci : 32 * ci + 32].rearrange(
                            "p (co q) -> p co q", co=4
                        ),
                        single_packet=store_sp,
                    )


def kernel(x: np.ndarray) -> np.ndarray:
    assert x.shape == (B, C, H, W), x.shape
    if "nc" not in _cache:
        _cache["nc"] = _build_nc()
    nc = _cache["nc"]
    x = np.ascontiguousarray(np.asarray(x, dtype=np.float32))
    in_maps = [{"x": x[b]} for b in range(N_CORES)]
    res = run_bass_kernel_spmd(nc, in_maps, core_ids=list(range(N_CORES)))
    return np.stack([res.results[b]["out"] for b in range(N_CORES)], axis=0)



# revision 8
# speedup vs baseline: 1.5639x; 1.5639x over previous
"""Trainium2 Bass kernel for space-to-depth (pixel-unshuffle, factor 2).

Input  x:   (8, 32, 512, 512) f32
Output out: (8, 128, 256, 256) f32 with out[b, 4i+2dh+dw, h, w] = x[b, i, 2h+dh, 2w+dw]

Sharding: data-parallel over batch -- core b processes sample b (no comms).

Per-core dataflow (v2): 8 groups of 4 input channels (4MB tiles).
  - one 4MB load per group (32KB contiguous per partition; SP HWDGE ring)
  - 4 strided DVE copies (one per (dh,dw) phase) deinterleave in SBUF;
    fully hidden behind DMA
  - 4 stores of 1MB per group (8KB contiguous runs; ACT HWDGE ring;
    single_packet=True bundles each engine's descriptors into one packet,
    coarsening R/W interleave -- won 3/4 paired A/Bs, mean +4.6us/op)
Measured on TRN2 (8 cores): ~203-210us exec; steady-state marginal rate
~190us/op = 354 GB/s combined R+W per core (pure-read 399, pure-write 374,
interleaved-mix 313 GB/s measured), ~8us framework preamble + ~3us tail.
"""

import numpy as np

from concourse import bacc, mybir, tile
from concourse.bass_utils import run_bass_kernel_spmd

B, C, H, W = 8, 32, 512, 512
N_CORES = 8

_cache = {}


def _build_nc(finalize=True, reps=1, variant="v2", bufs=(3, 2), store_engine="scalar", store_sp=True, load_sp=False, load_split=1):
    nc = bacc.Bacc(
        "TRN2", target_bir_lowering=False, debug=False, num_devices=N_CORES
    )
    x = nc.dram_tensor("x", [C, H, W], mybir.dt.float32, kind="ExternalInput")
    out = nc.dram_tensor(
        "out", [4 * C, H // 2, W // 2], mybir.dt.float32, kind="ExternalOutput"
    )
    xa, oa = x.ap(), out.ap()

    if variant == "raw":
        _emit_raw(nc, xa, oa, reps)
    else:
        with tile.TileContext(nc) as tc:
            if variant == "v1":
                _emit_v1(nc, tc, xa, oa, reps)
            elif variant == "v3":
                _emit_v3(nc, tc, xa, oa, reps, bufs, store_engine, store_sp, load_sp, load_split)
            else:
                _emit_v2(nc, tc, xa, oa, reps, bufs, store_engine, store_sp, load_sp)
    if finalize:
        nc.finalize()
    return nc


def _emit_v3(nc, tc, xa, oa, reps, bufs=(2, 4), store_engine="scalar", store_sp=True, load_sp=False, load_split=1):
    """8 channels per tile (8MB); per-channel DMAs with DRAM-side outer dim
    16 so every dma_start fans across all 16 SDMA engines.

    The engine fan of a dma_start follows the DRAM-side outer AP dim
    (min(outer, 16)): loads are one dma per (g, ci) with DRAM AP
    (16, 16384/load_split), stores one dma per (g, ci, co) writing one FULL
    output channel (256KB contiguous) with DRAM AP (16, 4096) -- 16 engines
    x one 16KB descriptor each.

    In-tile partition p=(ci=p>>4, pp=p&15) holds x[8g+ci, 32pp:32pp+32, :]
    (64KB contiguous).  Staging tile s_co (co=2dh+dw) partition p holds
    out[32g+4ci+co, 16pp:16pp+16, :] as one 16KB contiguous run.
    load_split subdivides each 1MB channel load along the free dim to
    shrink load packets (64KB/engine /split) for R/W service balance.
    """
    G = C // 8  # 4 groups
    if isinstance(bufs, int):
        bufs = (bufs, bufs)
    store_eng = getattr(nc, store_engine)
    # pp-major partition map: partition p = 8*pp + ci.  Both load and store
    # DRAM APs then have outer dim 16 (pp) -> 16-engine fan, and engine i's
    # descs walk partitions 8i..8i+8 = two alternating SBUF port blocks.
    oview = oa.rearrange(
        "(g ci co) (pp hh) w -> g co pp ci (hh w)", g=G, co=4, pp=16
    )
    with (
        tc.tile_pool(name="inp", bufs=bufs[0]) as ip,
        tc.tile_pool(name="stg", bufs=bufs[1]) as sp,
    ):
        for _ in range(reps):
            for g in range(G):
                t = ip.tile([128, 16384], mybir.dt.float32)
                src = xa[8 * g : 8 * g + 8].rearrange(
                    "ci (pp r) w -> pp ci (r w)", pp=16
                )
                if load_split == 1:
                    nc.sync.dma_start(t[:], src, single_packet=load_sp)
                else:
                    sv = src.rearrange("pp ci (ls q) -> ls pp ci q", ls=load_split)
                    tv = t[:].rearrange("p (ls q) -> ls p q", ls=load_split)
                    for j in range(load_split):
                        nc.sync.dma_start(tv[j], sv[j], single_packet=load_sp)
                t3 = t[:].rearrange("p (r w) -> p r w", r=32)
                for co in range(4):
                    dh, dw = co >> 1, co & 1
                    # staging partition 8pp+ci holds out[32g+4ci+co,
                    # 16pp:16pp+16, :] as one 16KB contiguous run
                    s = sp.tile([128, 4096], mybir.dt.float32)
                    nc.vector.tensor_copy(
                        s[:].rearrange("p (hh w) -> p hh w", hh=16),
                        t3[:, dh::2, dw::2],
                    )
                    store_eng.dma_start(
                        oview[g, co], s[:], single_packet=store_sp
                    )


def _emit_raw(nc, xa, oa, reps):
    """Same dataflow as v2 but raw bacc: hand-rolled semaphore pipeline,
    no TileContext, so the first load issues immediately instead of after
    the ~8us Tile preamble.  3 in-tile buffers, 2 staging buffers.
    """
    G = C // 4
    NB_IN, NB_ST = 3, 2
    tin = [
        nc.alloc_sbuf_tensor(f"tin{j}", [128, 8192], mybir.dt.float32)
        for j in range(NB_IN)
    ]
    tst = [
        nc.alloc_sbuf_tensor(f"tst{j}", [128, 8192], mybir.dt.float32)
        for j in range(NB_ST)
    ]
    n = G * reps
    from contextlib import ExitStack

    with ExitStack() as ctx:
        block = ctx.enter_context(nc.Block())
        # per-buffer sem rotation so concurrent DMAs never share a semaphore
        ld_sems = [
            ctx.enter_context(nc.semaphore(f"ld_sem{j}")) for j in range(NB_IN)
        ]
        st_sems = [
            ctx.enter_context(nc.semaphore(f"st_sem{j}")) for j in range(NB_ST)
        ]
        cp_sem = ctx.enter_context(nc.semaphore("cp_sem"))

        @block.sync
        def _(sync):
            for k in range(n):
                g = k % G
                if k >= NB_IN:
                    # in-buffer reuse: copies of group k-NB_IN must be done
                    sync.wait_ge(cp_sem, k - NB_IN + 1)
                sync.dma_start(
                    tin[k % NB_IN].ap(),
                    xa[4 * g : 4 * g + 4].rearrange(
                        "ci (pp r) w -> (ci pp) (r w)", pp=32
                    ),
                ).then_inc(ld_sems[k % NB_IN], 16)

        @block.vector
        def _(vec):
            for k in range(n):
                vec.wait_ge(ld_sems[k % NB_IN], 16 * (k // NB_IN + 1))
                if k >= NB_ST:
                    # staging reuse: stores of group k-NB_ST must be done
                    vec.wait_ge(st_sems[k % NB_ST], 64 * (k // NB_ST))
                t3 = tin[k % NB_IN].ap().rearrange("p (j w) -> p j w", j=16)
                s4 = (
                    tst[k % NB_ST]
                    .ap()
                    .rearrange("p (co hh w) -> p co hh w", co=4, hh=8)
                )
                last = None
                for dh in range(2):
                    for dw in range(2):
                        last = vec.tensor_copy(
                            s4[:, 2 * dh + dw], t3[:, dh::2, dw::2]
                        )
                last.then_inc(cp_sem, 1)

        @block.scalar
        def _(scalar):
            for k in range(n):
                g = k % G
                scalar.wait_ge(cp_sem, k + 1)
                s = tst[k % NB_ST].ap()
                for ci in range(4):
                    c0 = 16 * g + 4 * ci
                    scalar.dma_start(
                        oa[c0 : c0 + 4].rearrange(
                            "co (pp hh) w -> pp co (hh w)", hh=8
                        ),
                        s[32 * ci : 32 * ci + 32].rearrange(
                            "p (co q) -> p co q", co=4
                        ),
                    ).then_inc(st_sems[k % NB_ST], 16)


def _emit_v1(nc, tc, xa, oa, reps):
    """1 channel per tile: 1MB loads (8KB descs), 1MB stores (2KB descs)."""
    with (
        tc.tile_pool(name="inp", bufs=3) as ip,
        tc.tile_pool(name="stg", bufs=3) as sp,
    ):
        for _ in range(reps):
            for i in range(C):
                t = ip.tile([128, 2048], mybir.dt.float32)
                # partition p <- x[i, 4p:4p+4, :] (8KB contiguous per partition)
                nc.sync.dma_start(
                    t[:], xa[i].rearrange("(p r) w -> p (r w)", p=128)
                )
                s = sp.tile([128, 2048], mybir.dt.float32)
                t3 = t[:].rearrange("p (j w) -> p j w", j=4)
                s4 = s[:].rearrange("p (c hh w) -> p c hh w", c=4, hh=2)
                for dh in range(2):
                    for dw in range(2):
                        nc.vector.tensor_copy(
                            s4[:, 2 * dh + dw], t3[:, dh::2, dw::2]
                        )
                # staging partition p rows (2p, 2p+1) -> 2KB contiguous runs
                nc.sync.dma_start(
                    oa[4 * i : 4 * i + 4].rearrange(
                        "c (p hh) w -> p c (hh w)", p=128, hh=2
                    ),
                    s[:].rearrange("p (c q) -> p c q", c=4),
                )


def _emit_v2(nc, tc, xa, oa, reps, bufs, store_engine="scalar", store_sp=False, load_sp=False):
    """4 channels per tile (4MB): 8KB descriptors on BOTH load and store;
    loads on the SP HWDGE ring, stores on the ACT ring.

    Tile partition p = (ci=p>>5, pp=p&31) holds x[4g+ci, 16pp:16pp+16, :]
    (32KB contiguous).  Staging partition p holds, for each co in 0..3,
    out[4*(4g+ci)+co, 8pp:8pp+8, :] as one 8KB contiguous run.
    """
    G = C // 4  # 8 groups
    if isinstance(bufs, int):
        bufs = (bufs, bufs)
    store_eng = getattr(nc, store_engine)
    with (
        tc.tile_pool(name="inp", bufs=bufs[0]) as ip,
        tc.tile_pool(name="stg", bufs=bufs[1]) as sp,
    ):
        for _ in range(reps):
            for g in range(G):
                t = ip.tile([128, 8192], mybir.dt.float32)
                nc.sync.dma_start(
                    t[:],
                    xa[4 * g : 4 * g + 4].rearrange(
                        "ci (pp r) w -> (ci pp) (r w)", pp=32
                    ),
                    single_packet=load_sp,
                )
                s = sp.tile([128, 8192], mybir.dt.float32)
                t3 = t[:].rearrange("p (j w) -> p j w", j=16)
                s4 = s[:].rearrange("p (co hh w) -> p co hh w", co=4, hh=8)
                for dh in range(2):
                    for dw in range(2):
                        nc.vector.tensor_copy(
                            s4[:, 2 * dh + dw], t3[:, dh::2, dw::2]
                        )
                for ci in range(4):
                    c0 = 16 * g + 4 * ci
                    store_eng.dma_start(
                        oa[c0 : c0 + 4].rearrange(
                            "co (pp hh) w -> pp co (hh w)", hh=8
                        ),
                        s[32 * ci : 32 * ci + 32].rearrange(
                            "p (co q) -> p co q", co=4
                        ),
                        single_packet=store_sp,
                    )


def kernel(x: np.ndarray) -> np.ndarray:
    assert x.shape == (B, C, H, W), x.shape
    if "nc" not in _cache:
        _cache["nc"] = _build_nc()
    nc = _cache["nc"]
    x = np.ascontiguousarray(np.asarray(x, dtype=np.float32))
    in_maps = [{"x": x[b]} for b in range(N_CORES)]
    res = run_bass_kernel_spmd(nc, in_maps, core_ids=list(range(N_CORES)))
    return np.stack([res.results[b]["out"] for b in range(N_CORES)], axis=0)



# revision 10
# speedup vs baseline: 1.9462x; 1.2445x over previous
"""Trainium2 Bass kernel for space-to-depth (pixel-unshuffle, factor 2).

Input  x:   (8, 32, 512, 512) f32
Output out: (8, 128, 256, 256) f32 with out[b, 4i+2dh+dw, h, w] = x[b, i, 2h+dh, 2w+dw]

Sharding: data-parallel over batch -- core b processes sample b (no comms).

Per-core dataflow (v2): 8 groups of 4 input channels (4MB tiles).
  - one 4MB load per group (32KB contiguous per partition; SP HWDGE ring)
  - 4 strided DVE copies (one per (dh,dw) phase) deinterleave in SBUF;
    fully hidden behind DMA
  - 4 stores of 1MB per group (8KB contiguous runs; ACT HWDGE ring;
    single_packet=True bundles each engine's descriptors into one packet,
    coarsening R/W interleave -- won 3/4 paired A/Bs, mean +4.6us/op)
Measured on TRN2 (8 cores): ~203-210us exec; steady-state marginal rate
~190us/op = 354 GB/s combined R+W per core (pure-read 399, pure-write 374,
interleaved-mix 313 GB/s measured), ~8us framework preamble + ~3us tail.
"""

import numpy as np

from concourse import bacc, mybir, tile
from concourse.bass_utils import run_bass_kernel_spmd

B, C, H, W = 8, 32, 512, 512
N_CORES = 8

_cache = {}


def _build_nc(finalize=True, reps=1, variant="v2", bufs=(3, 2), store_engine="scalar", store_sp=True, load_sp=False, load_split=1):
    nc = bacc.Bacc(
        "TRN2", target_bir_lowering=False, debug=False, num_devices=N_CORES
    )
    x = nc.dram_tensor("x", [C, H, W], mybir.dt.float32, kind="ExternalInput")
    out = nc.dram_tensor(
        "out", [4 * C, H // 2, W // 2], mybir.dt.float32, kind="ExternalOutput"
    )
    xa, oa = x.ap(), out.ap()

    if variant == "raw":
        _emit_raw(nc, xa, oa, reps)
    else:
        with tile.TileContext(nc) as tc:
            if variant == "v1":
                _emit_v1(nc, tc, xa, oa, reps)
            elif variant == "v3":
                _emit_v3(nc, tc, xa, oa, reps, bufs, store_engine, store_sp, load_sp, load_split)
            elif variant == "v4":
                _emit_v4(nc, tc, xa, oa, reps, bufs, store_engine, store_sp, load_sp, load_split)
            else:
                _emit_v2(nc, tc, xa, oa, reps, bufs, store_engine, store_sp, load_sp)
    if finalize:
        nc.finalize()
    return nc


def _emit_v3(nc, tc, xa, oa, reps, bufs=(2, 4), store_engine="scalar", store_sp=True, load_sp=False, load_split=1):
    """8 channels per tile (8MB); per-channel DMAs with DRAM-side outer dim
    16 so every dma_start fans across all 16 SDMA engines.

    The engine fan of a dma_start follows the DRAM-side outer AP dim
    (min(outer, 16)): loads are one dma per (g, ci) with DRAM AP
    (16, 16384/load_split), stores one dma per (g, ci, co) writing one FULL
    output channel (256KB contiguous) with DRAM AP (16, 4096) -- 16 engines
    x one 16KB descriptor each.

    In-tile partition p=(ci=p>>4, pp=p&15) holds x[8g+ci, 32pp:32pp+32, :]
    (64KB contiguous).  Staging tile s_co (co=2dh+dw) partition p holds
    out[32g+4ci+co, 16pp:16pp+16, :] as one 16KB contiguous run.
    load_split subdivides each 1MB channel load along the free dim to
    shrink load packets (64KB/engine /split) for R/W service balance.
    """
    G = C // 8  # 4 groups
    if isinstance(bufs, int):
        bufs = (bufs, bufs)
    store_eng = getattr(nc, store_engine)
    # pp-major partition map: partition p = 8*pp + ci.  Both load and store
    # DRAM APs then have outer dim 16 (pp) -> 16-engine fan, and engine i's
    # descs walk partitions 8i..8i+8 = two alternating SBUF port blocks.
    oview = oa.rearrange(
        "(g ci co) (pp hh) w -> g co pp ci (hh w)", g=G, co=4, pp=16
    )
    with (
        tc.tile_pool(name="inp", bufs=bufs[0]) as ip,
        tc.tile_pool(name="stg", bufs=bufs[1]) as sp,
    ):
        for _ in range(reps):
            for g in range(G):
                t = ip.tile([128, 16384], mybir.dt.float32)
                src = xa[8 * g : 8 * g + 8].rearrange(
                    "ci (pp r) w -> pp ci (r w)", pp=16
                )
                if load_split == 1:
                    nc.sync.dma_start(t[:], src, single_packet=load_sp)
                else:
                    sv = src.rearrange("pp ci (ls q) -> ls pp ci q", ls=load_split)
                    tv = t[:].rearrange("p (ls q) -> ls p q", ls=load_split)
                    for j in range(load_split):
                        nc.sync.dma_start(tv[j], sv[j], single_packet=load_sp)
                t3 = t[:].rearrange("p (r w) -> p r w", r=32)
                for co in range(4):
                    dh, dw = co >> 1, co & 1
                    # staging partition 8pp+ci holds out[32g+4ci+co,
                    # 16pp:16pp+16, :] as one 16KB contiguous run
                    s = sp.tile([128, 4096], mybir.dt.float32)
                    nc.vector.tensor_copy(
                        s[:].rearrange("p (hh w) -> p hh w", hh=16),
                        t3[:, dh::2, dw::2],
                    )
                    store_eng.dma_start(
                        oview[g, co], s[:], single_packet=store_sp
                    )


def _emit_raw(nc, xa, oa, reps):
    """Same dataflow as v2 but raw bacc: hand-rolled semaphore pipeline,
    no TileContext, so the first load issues immediately instead of after
    the ~8us Tile preamble.  3 in-tile buffers, 2 staging buffers.
    """
    G = C // 4
    NB_IN, NB_ST = 3, 2
    tin = [
        nc.alloc_sbuf_tensor(f"tin{j}", [128, 8192], mybir.dt.float32)
        for j in range(NB_IN)
    ]
    tst = [
        nc.alloc_sbuf_tensor(f"tst{j}", [128, 8192], mybir.dt.float32)
        for j in range(NB_ST)
    ]
    n = G * reps
    from contextlib import ExitStack

    with ExitStack() as ctx:
        block = ctx.enter_context(nc.Block())
        # per-buffer sem rotation so concurrent DMAs never share a semaphore
        ld_sems = [
            ctx.enter_context(nc.semaphore(f"ld_sem{j}")) for j in range(NB_IN)
        ]
        st_sems = [
            ctx.enter_context(nc.semaphore(f"st_sem{j}")) for j in range(NB_ST)
        ]
        cp_sem = ctx.enter_context(nc.semaphore("cp_sem"))

        @block.sync
        def _(sync):
            for k in range(n):
                g = k % G
                if k >= NB_IN:
                    # in-buffer reuse: copies of group k-NB_IN must be done
                    sync.wait_ge(cp_sem, k - NB_IN + 1)
                sync.dma_start(
                    tin[k % NB_IN].ap(),
                    xa[4 * g : 4 * g + 4].rearrange(
                        "ci (pp r) w -> (ci pp) (r w)", pp=32
                    ),
                ).then_inc(ld_sems[k % NB_IN], 16)

        @block.vector
        def _(vec):
            for k in range(n):
                vec.wait_ge(ld_sems[k % NB_IN], 16 * (k // NB_IN + 1))
                if k >= NB_ST:
                    # staging reuse: stores of group k-NB_ST must be done
                    vec.wait_ge(st_sems[k % NB_ST], 64 * (k // NB_ST))
                t3 = tin[k % NB_IN].ap().rearrange("p (j w) -> p j w", j=16)
                s4 = (
                    tst[k % NB_ST]
                    .ap()
                    .rearrange("p (co hh w) -> p co hh w", co=4, hh=8)
                )
                last = None
                for dh in range(2):
                    for dw in range(2):
                        last = vec.tensor_copy(
                            s4[:, 2 * dh + dw], t3[:, dh::2, dw::2]
                        )
                last.then_inc(cp_sem, 1)

        @block.scalar
        def _(scalar):
            for k in range(n):
                g = k % G
                scalar.wait_ge(cp_sem, k + 1)
                s = tst[k % NB_ST].ap()
                for ci in range(4):
                    c0 = 16 * g + 4 * ci
                    scalar.dma_start(
                        oa[c0 : c0 + 4].rearrange(
                            "co (pp hh) w -> pp co (hh w)", hh=8
                        ),
                        s[32 * ci : 32 * ci + 32].rearrange(
                            "p (co q) -> p co q", co=4
                        ),
                    ).then_inc(st_sems[k % NB_ST], 16)


def _emit_v1(nc, tc, xa, oa, reps):
    """1 channel per tile: 1MB loads (8KB descs), 1MB stores (2KB descs)."""
    with (
        tc.tile_pool(name="inp", bufs=3) as ip,
        tc.tile_pool(name="stg", bufs=3) as sp,
    ):
        for _ in range(reps):
            for i in range(C):
                t = ip.tile([128, 2048], mybir.dt.float32)
                # partition p <- x[i, 4p:4p+4, :] (8KB contiguous per partition)
                nc.sync.dma_start(
                    t[:], xa[i].rearrange("(p r) w -> p (r w)", p=128)
                )
                s = sp.tile([128, 2048], mybir.dt.float32)
                t3 = t[:].rearrange("p (j w) -> p j w", j=4)
                s4 = s[:].rearrange("p (c hh w) -> p c hh w", c=4, hh=2)
                for dh in range(2):
                    for dw in range(2):
                        nc.vector.tensor_copy(
                            s4[:, 2 * dh + dw], t3[:, dh::2, dw::2]
                        )
                # staging partition p rows (2p, 2p+1) -> 2KB contiguous runs
                nc.sync.dma_start(
                    oa[4 * i : 4 * i + 4].rearrange(
                        "c (p hh) w -> p c (hh w)", p=128, hh=2
                    ),
                    s[:].rearrange("p (c q) -> p c q", c=4),
                )


def _emit_v2(nc, tc, xa, oa, reps, bufs, store_engine="scalar", store_sp=False, load_sp=False):
    """4 channels per tile (4MB): 8KB descriptors on BOTH load and store;
    loads on the SP HWDGE ring, stores on the ACT ring.

    Tile partition p = (ci=p>>5, pp=p&31) holds x[4g+ci, 16pp:16pp+16, :]
    (32KB contiguous).  Staging partition p holds, for each co in 0..3,
    out[4*(4g+ci)+co, 8pp:8pp+8, :] as one 8KB contiguous run.
    """
    G = C // 4  # 8 groups
    if isinstance(bufs, int):
        bufs = (bufs, bufs)
    store_eng = getattr(nc, store_engine)
    with (
        tc.tile_pool(name="inp", bufs=bufs[0]) as ip,
        tc.tile_pool(name="stg", bufs=bufs[1]) as sp,
    ):
        for _ in range(reps):
            for g in range(G):
                t = ip.tile([128, 8192], mybir.dt.float32)
                nc.sync.dma_start(
                    t[:],
                    xa[4 * g : 4 * g + 4].rearrange(
                        "ci (pp r) w -> (ci pp) (r w)", pp=32
                    ),
                    single_packet=load_sp,
                )
                s = sp.tile([128, 8192], mybir.dt.float32)
                t3 = t[:].rearrange("p (j w) -> p j w", j=16)
                s4 = s[:].rearrange("p (co hh w) -> p co hh w", co=4, hh=8)
                for dh in range(2):
                    for dw in range(2):
                        nc.vector.tensor_copy(
                            s4[:, 2 * dh + dw], t3[:, dh::2, dw::2]
                        )
                for ci in range(4):
                    c0 = 16 * g + 4 * ci
                    store_eng.dma_start(
                        oa[c0 : c0 + 4].rearrange(
                            "co (pp hh) w -> pp co (hh w)", hh=8
                        ),
                        s[32 * ci : 32 * ci + 32].rearrange(
                            "p (co q) -> p co q", co=4
                        ),
                        single_packet=store_sp,
                    )


def _emit_v4(nc, tc, xa, oa, reps, bufs=(2, 4), store_engine="scalar", store_sp=True, load_sp=False, load_split=1):
    """16-channel x half-height slabs (8MB).  Both load and store DRAM APs
    are (ci=16, pp=8, inner): outer dim 16 -> 16-engine fan, and each
    engine's 8 descriptors walk a CONTIGUOUS DRAM range (512KB load run /
    128KB store run).  Loads: 64KB descs; stores: 16KB descs (~610ns each,
    port speed).  Empirically: per-descriptor rate caps at ~27GB/s (single
    SBUF port); loads additionally need sequential per-engine HBM walks.

    Slab (gg, hf) = x[16gg:16gg+16, 256hf:256hf+256, :].  In-tile partition
    p=(ci=p>>3, pp=p&7) holds x[16gg+ci, 256hf+32pp : +32, :] (64KB).
    Staging s_co partition p holds out[64gg+4ci+co, 128hf+16pp : +16, :]
    (16KB contiguous).
    """
    if isinstance(bufs, int):
        bufs = (bufs, bufs)
    store_eng = getattr(nc, store_engine)
    with (
        tc.tile_pool(name="inp", bufs=bufs[0]) as ip,
        tc.tile_pool(name="stg", bufs=bufs[1]) as sp,
    ):
        for _ in range(reps):
            for gg in range(C // 16):
                for hf in range(2):
                    t = ip.tile([128, 16384], mybir.dt.float32)
                    src = xa[
                        16 * gg : 16 * gg + 16, 256 * hf : 256 * hf + 256
                    ].rearrange("ci (pp r) w -> ci pp (r w)", pp=8)
                    if load_split == 1:
                        nc.sync.dma_start(t[:], src, single_packet=load_sp)
                    else:
                        sv = src.rearrange(
                            "ci pp (ls q) -> ls ci pp q", ls=load_split
                        )
                        tv = t[:].rearrange("p (ls q) -> ls p q", ls=load_split)
                        for j in range(load_split):
                            nc.sync.dma_start(tv[j], sv[j], single_packet=load_sp)
                    t3 = t[:].rearrange("p (r w) -> p r w", r=32)
                    ov = oa[
                        64 * gg : 64 * gg + 64, 128 * hf : 128 * hf + 128
                    ].rearrange("(ci co) (pp hh) w -> co ci pp (hh w)", co=4, pp=8)
                    for co in range(4):
                        dh, dw = co >> 1, co & 1
                        s = sp.tile([128, 4096], mybir.dt.float32)
                        nc.vector.tensor_copy(
                            s[:].rearrange("p (hh w) -> p hh w", hh=16),
                            t3[:, dh::2, dw::2],
                        )
                        store_eng.dma_start(
                            ov[co], s[:], single_packet=store_sp
                        )


def kernel(x: np.ndarray) -> np.ndarray:
    assert x.shape == (B, C, H, W), x.shape
    if "nc" not in _cache:
        _cache["nc"] = _build_nc()
    nc = _cache["nc"]
    x = np.ascontiguousarray(np.asarray(x, dtype=np.float32))
    in_maps = [{"x": x[b]} for b in range(N_CORES)]
    res = run_bass_kernel_spmd(nc, in_maps, core_ids=list(range(N_CORES)))
    return np.stack([res.results[b]["out"] for b in range(N_CORES)], axis=0)

